# revision 2
# baseline (speedup 1.0000x reference)
"""Causal self-attention (B=4, T=2048, C=1024, H=16) on 8 TRN2 NeuronCores.

Sharding: core = 2*b + g (b = batch 0..3, g = head-group 0..1). Each core
computes qkv + attention for its batch and its 8 heads, then a PARTIAL
out-projection (contraction over its 512 y-columns) across the FULL output;
the host sums the two partials per batch. No collectives.

Key structure (all f16 operands, f32 PSUM accumulation):
- x / weights DMA'd once, SBUF-resident; ~55 DMAs total.
- QK^T: stationary k-tile [64, 128], moving q [64, 512] -> scores [kpos, q].
- exp on Act engine (no max-subtraction; logits are O(1) after 0.125 scale),
  f16 probabilities; causal mask = one 128-wide multiply per diagonal tile.
- PV streams V: stationary p-tile [128, 128], moving v|1 [128, 65]
  -> y [q, dh] at 65 cycles per score-tile (vs 512 the other orientation),
  with the softmax denominator riding the ones column.
- normalize: per-partition reciprocal + tensor_scalar multiply (q is the
  partition dim), then PE-transpose into y^T [c, q] for the out-projection.
- c-major software pipeline: scores(item) | pv(prev) | transposes(prevprev)
  interleaved with qkv-proj / out-proj filler to keep PE busy.
"""
import numpy as np

D_MODEL = 1024
N_HEAD = 16
D_HEAD = 64
B = 4
T = 2048
N_CORES = 8
P = 128
PAIRS = 4          # head pairs per core
KT = D_MODEL // P  # 8 contraction tiles
NQ = 4             # q-chunks of 512
QC = 512           # q chunk width

_RUNNER_CACHE = {}


def _build(has_qk_bias: bool):
    from concourse import bacc
    import concourse.mybir as mybir
    from concourse.tile import TileContext
    from concourse.bass import ts

    f32 = mybir.dt.float32
    f16 = mybir.dt.float16
    KD = D_MODEL + (1 if has_qk_bias else 0)

    nc = bacc.Bacc("TRN2", target_bir_lowering=False, debug=False,
                   num_devices=N_CORES)
    xT = nc.dram_tensor("xT", [KD, T], f16, kind="ExternalInput")
    wqk = nc.dram_tensor("wqk", [KD, 1024], f16, kind="ExternalInput")
    wv = nc.dram_tensor("wv", [D_MODEL, 512], f16, kind="ExternalInput")
    wp = nc.dram_tensor("wp", [512, 1024], f16, kind="ExternalInput")
    tri2 = nc.dram_tensor("tri2", [P, 2 * P], f16, kind="ExternalInput")
    ident = nc.dram_tensor("ident", [P, P], f16, kind="ExternalInput")
    out = nc.dram_tensor("out", [T, 1024], f16, kind="ExternalOutput")

    with TileContext(nc) as tc:
        with (
            tc.tile_pool(name="const", bufs=1) as cp,
            tc.tile_pool(name="qk_res", bufs=1) as qk_res,
            tc.tile_pool(name="v_res", bufs=1) as v_res,
            tc.tile_pool(name="yf_res", bufs=1) as yf_res,
            tc.tile_pool(name="ex", bufs=24) as ex_pool,
            tc.tile_pool(name="ys", bufs=18) as ys_pool,
            tc.tile_pool(name="rc", bufs=18) as rc_pool,
            tc.tile_pool(name="ob", bufs=4) as o_pool,
            tc.tile_pool(name="stp", bufs=2, space="PSUM") as st_pool,
            tc.tile_pool(name="pjp", bufs=2, space="PSUM") as pj_pool,
            tc.tile_pool(name="yp", bufs=2, space="PSUM") as y_pool,
        ):
            # ---------------- constants / inputs ----------------
            x_sb = cp.tile([P, KT, T], f16, name="x_sb")
            wqk_sb = cp.tile([P, KT, 1024], f16, name="wqk_sb")
            wv_sb = cp.tile([P, KT, 512], f16, name="wv_sb")
            wp_sb = cp.tile([P, 4, 1024], f16, name="wp_sb")
            tri_sb = cp.tile([P, 2, P], f16, name="tri_sb")
            id_sb = cp.tile([P, P], f16, name="id_sb")
            # DMA order tuned so vproj can start after wv + x-chunk0 and
            # qkproj(0) is never starved: id, wv, x0, wqk, x1..x3, wp, tri.
            nc.sync.dma_start(out=id_sb, in_=ident[:])
            nc.sync.dma_start(
                out=wv_sb, in_=wv[:].rearrange("(k p) c -> p k c", p=P))
            nc.sync.dma_start(
                out=x_sb[:, :, ts(0, QC)],
                in_=xT[0:D_MODEL, ts(0, QC)].rearrange("(k p) t -> p k t",
                                                       p=P))
            nc.sync.dma_start(
                out=wqk_sb,
                in_=wqk[0:D_MODEL, :].rearrange("(k p) c -> p k c", p=P))
            for nn in range(1, NQ):
                nc.sync.dma_start(
                    out=x_sb[:, :, ts(nn, QC)],
                    in_=xT[0:D_MODEL, ts(nn, QC)].rearrange("(k p) t -> p k t",
                                                            p=P))
            nc.sync.dma_start(
                out=wp_sb, in_=wp[:].rearrange("(k p) c -> p k c", p=P))
            nc.sync.dma_start(out=tri_sb, in_=tri2[:].rearrange(
                "p (j c) -> p j c", j=2))
            if has_qk_bias:
                xrow = cp.tile([1, T], f16, name="xrow")
                nc.sync.dma_start(out=xrow, in_=xT[D_MODEL:D_MODEL + 1, :])
                wrow = cp.tile([1, 1024], f16, name="wrow")
                nc.sync.dma_start(out=wrow, in_=wqk[D_MODEL:D_MODEL + 1, :])

            qT = [qk_res.tile([P, T], f16, name=f"qT{p}") for p in range(PAIRS)]
            kT = [qk_res.tile([P, T], f16, name=f"kT{p}") for p in range(PAIRS)]
            v_sb = [v_res.tile([P, 8, 65], f16, name=f"v{t}")
                    for t in range(T // P)]
            yf = [yf_res.tile([P, T], f16, name=f"yf{p}") for p in range(PAIRS)]

            # ---------------- emit helpers ----------------
            def vproj(tt):
                """V projection for t-tile tt: v_sb[tt] <- x_tile @ wv."""
                ps = pj_pool.tile([P, 512], f32, name="vps", tag="pj")
                n, tl = divmod(tt, 4)
                for k in range(KT):
                    nc.tensor.matmul(ps[:], x_sb[:, k, ts(n, QC)][:, ts(tl, P)],
                                     wv_sb[:, k, :],
                                     start=(k == 0), stop=(k == KT - 1))
                nc.vector.memset(v_sb[tt][:, :, 64:65], 1.0)
                src = ps.rearrange("p (h c) -> p h c", c=64)
                nc.vector.tensor_copy(out=v_sb[tt][:, :, 0:64], in_=src[:])

            def qkproj_nm(p, n, m):
                dest = qT[p] if m == 0 else kT[p]
                ps = pj_pool.tile([P, 512], f32, name="qkps", tag="pj")
                for k in range(KT):
                    nc.tensor.matmul(
                        ps[:],
                        wqk_sb[:, k, p * 256 + m * P:p * 256 + (m + 1) * P],
                        x_sb[:, k, ts(n, QC)],
                        start=(k == 0),
                        stop=(k == KT - 1) and not has_qk_bias)
                if has_qk_bias:
                    nc.tensor.matmul(
                        ps[:],
                        wrow[:, p * 256 + m * P:p * 256 + (m + 1) * P],
                        xrow[:, ts(n, QC)],
                        start=False, stop=True)
                nc.vector.tensor_copy(out=dest[:, ts(n, QC)], in_=ps[:])

            def qkproj(p):
                """q/k projection for pair p -> qT[p], kT[p] (f16)."""
                for n in range(NQ):
                    for m in (0, 1):
                        qkproj_nm(p, n, m)

            def scores(p, c, filler=()):
                """QK^T + exp + causal mask for (pair p, q-chunk c).
                Emits one unit from `filler` (a list of closures) after each
                k-tile group so PE has independent queued work while the
                2-deep st ring throttles QK^T to the Act exp cadence.
                Returns ex-tile refs for pv()."""
                filler = list(filler)

                def fill(n=1):
                    for _ in range(n):
                        if filler:
                            filler.pop(0)()

                exs = [[None] * (2 * c) for _ in range(2)]  # [h][g]
                exd = [None] * 4                            # diagonal, h-stacked
                for h in (0, 1):
                    pb = h * 64
                    for g in range(2 * c):
                        st = st_pool.tile([P, 2, QC], f32, name="st", tag="st")
                        for j, kt in enumerate((2 * g, 2 * g + 1)):
                            nc.tensor.matmul(st[:, j, :],
                                             kT[p][pb:pb + 64, ts(kt, P)],
                                             qT[p][pb:pb + 64, ts(c, QC)],
                                             start=True, stop=True)
                        ex = ex_pool.tile([P, 2, QC], f16, name="ex", tag="ex")
                        nc.scalar.activation(
                            ex[:], st[:], mybir.ActivationFunctionType.Exp,
                            scale=0.125)
                        exs[h][g] = ex
                        fill()
                # diagonal tiles, h-stacked; d1 and d3 share one st tile
                # ([0:384] and [384:512]) so their exp is a single
                # instruction. start/stop flags per psum zero-region.
                for ds in ((0,), (1, 3), (2,)):
                    kt0 = 4 * c + ds[0]
                    st = st_pool.tile([P, 2, QC], f32, name="std", tag="st")
                    off = 0
                    offs = []
                    for d in ds:
                        offs.append(off)
                        w = QC - P * d
                        for h in (0, 1):
                            pb = h * 64
                            nc.tensor.matmul(
                                st[:, h, off:off + w],
                                kT[p][pb:pb + 64, ts(4 * c + d, P)],
                                qT[p][pb:pb + 64,
                                      c * QC + P * d:(c + 1) * QC],
                                start=(d == ds[0]), stop=(d == ds[-1]))
                        off += w
                    ex = ex_pool.tile([P, 2, QC], f16, name="exd", tag="ex")
                    nc.scalar.activation(
                        ex[:, :, 0:off], st[:, :, 0:off],
                        mybir.ActivationFunctionType.Exp, scale=0.125)
                    # causal mask on each 128-wide diagonal block; on Pool so
                    # DVE (yf copies feeding outproj) never queues behind an
                    # Act-dependent op
                    for d, off in zip(ds, offs):
                        nc.gpsimd.tensor_mul(ex[:, :, off:off + P],
                                             ex[:, :, off:off + P], tri_sb[:])
                        exd[d] = (ex, off)
                    fill()
                fill(len(filler))
                return exs, exd

            def pv_norm_units(p, c, exs, exd, ys_all):
                """PV + normalize closures for (p, c); executing all of them
                (in order) fills ys_all[h][qt_rel] with f16 tiles."""
                units = []
                state = {}
                CH = 8
                for h in (0, 1):
                    lh = 2 * p + h
                    mms = []
                    for kt in range(4 * c + 4):
                        for qt_rel in range(4):
                            qlo = qt_rel * P
                            if kt < 4 * c:
                                exsl = exs[h][kt // 2][:, kt % 2, qlo:qlo + P]
                            else:
                                d = kt - 4 * c
                                so = P * d
                                if qlo < so:
                                    continue
                                exd_t, exd_o = exd[d]
                                exsl = exd_t[:, h,
                                             exd_o + qlo - so:
                                             exd_o + qlo - so + P]
                            mms.append((qt_rel, exsl, v_sb[kt][:, lh, :]))
                    nch = (len(mms) + CH - 1) // CH

                    def pvu(h=h, ci=None, mms=mms, nch=nch):
                        if ci == 0:
                            state[h] = y_pool.tile([P, 4, 65], f32,
                                                   name="y_ps", tag="y")
                        y_ps = state[h]
                        for i in range(ci * CH, min((ci + 1) * CH, len(mms))):
                            qt_rel, exsl, vsl = mms[i]
                            nc.tensor.matmul(y_ps[:, qt_rel, :], exsl, vsl,
                                             start=(i == 0),
                                             stop=(i == len(mms) - 1))
                        if ci == nch - 1:
                            rc = rc_pool.tile([P, 4], f32, name="rc", tag="rc")
                            nc.vector.reciprocal(out=rc[:],
                                                 in_=y_ps[:, :, 64:65])
                            state[(h, 'rc')] = rc
                    for ci in range(nch):
                        units.append(lambda h=h, ci=ci, f=pvu: f(ci=ci))

                def normu(qts):
                    for qt_rel in qts:
                        for h in (0, 1):
                            yst = ys_pool.tile([P, 64], f16, name="ys",
                                               tag="ys")
                            nc.vector.tensor_scalar(
                                out=yst[:], in0=state[h][:, qt_rel, 0:64],
                                scalar1=state[(h, 'rc')][:, qt_rel:qt_rel + 1],
                                scalar2=None, op0=mybir.AluOpType.mult)
                            ys_all[h][qt_rel] = yst
                units.append(lambda: normu((0, 1)))
                units.append(lambda: normu((2, 3)))
                return units

            def trans_copy_units(p, c, ys_all):
                """Transpose ys -> y^T psum, assemble yf[p] (f16)."""
                def u(qt_rel):
                    qt = 4 * c + qt_rel
                    yt = y_pool.tile([P, P], f16, name="yt", tag="y")
                    nc.tensor.transpose(yt[0:64, :], ys_all[0][qt_rel], id_sb)
                    nc.tensor.transpose(yt[64:128, :], ys_all[1][qt_rel],
                                        id_sb)
                    nc.vector.tensor_copy(out=yf[p][:, ts(qt, P)], in_=yt[:])
                return [lambda qt_rel=q: u(qt_rel) for q in range(4)]

            def outproj_tt(tt):
                """Partial out-projection for one t-tile (needs all yf)."""
                for oc in (0, 1):
                    ps = pj_pool.tile([P, 512], f32, name="ops", tag="pj")
                    for ct in range(4):
                        nc.tensor.matmul(
                            ps[:], yf[ct][:, ts(tt, P)],
                            wp_sb[:, ct, ts(oc, 512)],
                            start=(ct == 0), stop=(ct == 3))
                    ot = o_pool.tile([P, 512], f16, name="ot", tag="ot")
                    nc.vector.tensor_copy(out=ot[:], in_=ps[:])
                    nc.sync.dma_start(out=out[ts(tt, P), ts(oc, 512)],
                                      in_=ot[:])

            def outproj(c):
                for tt in range(4 * c, 4 * c + 4):
                    outproj_tt(tt)

            # ---------------- schedule (c-major, 2-deep pipeline) ---------
            # PE warmup during the input-DMA head: ramps the p-state clock
            # (0.65 -> 2.4 GHz needs 3us of continuous busy) on zero-matmuls
            # with no DMA dependency.
            zw = cp.tile([P, QC], f16, name="zw")
            nc.vector.memset(zw, 0.0)
            warm = pj_pool.tile([P, QC], f32, name="warm", tag="pj")
            for i in range(22):
                nc.tensor.matmul(warm[:], zw[:, 0:P], zw[:],
                                 start=True, stop=True)

            def qk_units(p):
                return [(lambda n=n, m=m: qkproj_nm(p, n, m))
                        for n in range(NQ) for m in (0, 1)]

            def vp_unit(tt):
                return lambda: vproj(tt)

            def op_unit(tt):
                return lambda: outproj_tt(tt)

            # c-major item order; filler routed so the Act-bound c2/c3
            # stretches get the deferred out-projections as PE work:
            #   c0: qkproj(p+1) + vproj(4..7)
            #   c1 (p3): vproj(8..11)
            #   c2: vproj(12..15) + outproj(0), one of each per pair
            #   c3: outproj(1) and outproj(2), two per pair
            #   tail: outproj(3)
            for tt in range(4):
                vproj(tt)
            qkproj(0)
            post_map = {}
            for p in range(3):
                post_map[p] = qk_units(p + 1) + [vp_unit(4 + p)]
            post_map[3] = [vp_unit(7)]
            post_map[7] = [vp_unit(t) for t in range(8, 12)]
            # outproj(c) units may only appear after trans_copy((3, c)),
            # which trails item (3, c) by two pipeline slots.
            post_map[8] = [vp_unit(12)]
            post_map[9] = [vp_unit(13), op_unit(0)]
            post_map[10] = [op_unit(1), op_unit(2)]
            post_map[11] = [vp_unit(14), vp_unit(15), op_unit(3)]
            post_map[12] = [op_unit(4)]
            post_map[13] = [op_unit(5), op_unit(8)]
            post_map[14] = [op_unit(6), op_unit(9)]
            post_map[15] = [op_unit(7), op_unit(10)]
            prev = prevprev = None     # (p, c, exs, exd) / (p, c, ys_all)
            items = [(p, c) for c in range(NQ) for p in range(PAIRS)]
            for i, (p, c) in enumerate(items):
                filler = []
                if prevprev is not None:
                    qp, qc2, qys = prevprev
                    filler += trans_copy_units(qp, qc2, qys)
                    prevprev = None
                if prev is not None:
                    pp, pc, pexs, pexd = prev
                    ys_all = [[None] * 4 for _ in range(2)]
                    filler += pv_norm_units(pp, pc, pexs, pexd, ys_all)
                    prevprev = (pp, pc, ys_all)
                filler += post_map.get(i, [])
                sc = scores(p, c, filler)
                prev = (p, c, sc[0], sc[1])
            # ----- tail: finish last two items, interleave final outproj ----
            outproj_tt(11)   # runs while the final exp burst drains
            qp, qc2, qys = prevprev
            for u in trans_copy_units(qp, qc2, qys):
                u()
            pp, pc, pexs, pexd = prev
            ys_all = [[None] * 4 for _ in range(2)]
            for u in pv_norm_units(pp, pc, pexs, pexd, ys_all):
                u()
            tail_trans = trans_copy_units(pp, pc, ys_all)
            for qt_rel in range(4):
                tail_trans[qt_rel]()
                outproj_tt(12 + qt_rel)

    nc.compile()
    return nc


def _make_runner(nc):
    """Reusable 8-core SPMD runner (jit built once)."""
    import jax
    from jax.sharding import Mesh, PartitionSpec
    from jax.experimental.shard_map import shard_map
    from concourse import bass2jax
    import concourse.mybir as mybir

    bass2jax.install_neuronx_cc_hook()
    partition_name = (nc.partition_id_tensor.name
                      if nc.partition_id_tensor else None)
    in_names, out_names, out_avals, zero_outs = [], [], [], []
    for alloc in nc.m.functions[0].allocations:
        if not isinstance(alloc, mybir.MemoryLocationSet):
            continue
        name = alloc.memorylocations[0].name
        if alloc.kind == "ExternalInput":
            if name != partition_name:
                in_names.append(name)
        elif alloc.kind == "ExternalOutput":
            shape = tuple(alloc.tensor_shape)
            dtype = mybir.dt.np(alloc.dtype)
            out_names.append(name)
            out_avals.append(jax.core.ShapedArray(shape, dtype))
            zero_outs.append(np.zeros(shape, dtype))
    n_params = len(in_names)
    n_outs = len(out_avals)
    all_in = list(in_names) + list(out_names)
    if partition_name is not None:
        all_in.append(partition_name)

    def _body(*args):
        operands = list(args)
        if partition_name is not None:
            operands.append(bass2jax.partition_id_tensor())
        outs = bass2jax._bass_exec_p.bind(
            *operands,
            out_avals=tuple(out_avals),
            in_names=tuple(all_in),
            out_names=tuple(out_names),
            lowering_input_output_aliases=(),
            sim_require_finite=True,
            sim_require_nnan=True,
            nc=nc,
        )
        return tuple(outs)

    devices = jax.devices()[:N_CORES]
    mesh = Mesh(np.asarray(devices), ("core",))
    in_specs = (PartitionSpec("core"),) * (n_params + n_outs)
    out_specs = (PartitionSpec("core"),) * n_outs
    donate = tuple(range(n_params, n_params + n_outs))
    sharded = jax.jit(
        shard_map(_body, mesh=mesh, in_specs=in_specs, out_specs=out_specs,
                  check_rep=False),
        donate_argnums=donate, keep_unused=True)

    def run(in_maps):
        per_core = [[np.asarray(m[k]) for k in in_names] for m in in_maps]
        concat_in = [
            np.concatenate([per_core[c][i] for c in range(N_CORES)], axis=0)
            for i in range(n_params)]
        concat_zeros = [
            np.zeros((N_CORES * z.shape[0], *z.shape[1:]), z.dtype)
            for z in zero_outs]
        outs = sharded(*concat_in, *concat_zeros)
        jax.block_until_ready(outs)
        return [
            {name: np.asarray(outs[i]).reshape(N_CORES, *out_avals[i].shape)[c]
             for i, name in enumerate(out_names)}
            for c in range(N_CORES)]

    return run


def kernel(x, w_qkv, b_qkv, w_proj, b_proj):
    x = np.asarray(x, dtype=np.float32)
    w_qkv = np.asarray(w_qkv, dtype=np.float32)
    b_qkv = np.asarray(b_qkv, dtype=np.float32)
    w_proj = np.asarray(w_proj, dtype=np.float32)
    b_proj = np.asarray(b_proj, dtype=np.float32)

    w_q, w_k, w_v = w_qkv[0:1024], w_qkv[1024:2048], w_qkv[2048:3072]
    b_q, b_k, b_v = b_qkv[0:1024], b_qkv[1024:2048], b_qkv[2048:3072]
    has_qk_bias = bool(np.any(b_q) or np.any(b_k))

    key = ("runner", has_qk_bias)
    if key not in _RUNNER_CACHE:
        nc = _build(has_qk_bias)
        _RUNNER_CACHE[key] = _make_runner(nc)
    run = _RUNNER_CACHE[key]

    # causal block mask: tri2[k, j*128 + q] = 1.0 iff k <= q
    kk = np.arange(P)[:, None]
    qq = np.arange(P)[None, :]
    blk = (kk <= qq).astype(np.float16)
    tri2 = np.concatenate([blk, blk], axis=1)
    ident = np.eye(P, dtype=np.float16)

    in_maps = []
    for core in range(N_CORES):
        b, g = divmod(core, 2)
        xT_c = np.ascontiguousarray(x[b].T).astype(np.float16)
        if has_qk_bias:
            xT_c = np.concatenate([xT_c, np.ones((1, T), np.float16)], axis=0)
        wqk_c = np.empty((D_MODEL + (1 if has_qk_bias else 0), 1024),
                         np.float16)
        for p in range(PAIRS):
            hA = 8 * g + 2 * p
            hB = hA + 1
            cols = p * 256
            wqk_c[:D_MODEL, cols + 0:cols + 64] = w_q[hA * 64:(hA + 1) * 64].T
            wqk_c[:D_MODEL, cols + 64:cols + 128] = w_q[hB * 64:(hB + 1) * 64].T
            wqk_c[:D_MODEL, cols + 128:cols + 192] = w_k[hA * 64:(hA + 1) * 64].T
            wqk_c[:D_MODEL, cols + 192:cols + 256] = w_k[hB * 64:(hB + 1) * 64].T
            if has_qk_bias:
                wqk_c[D_MODEL, cols + 0:cols + 64] = b_q[hA * 64:(hA + 1) * 64]
                wqk_c[D_MODEL, cols + 64:cols + 128] = b_q[hB * 64:(hB + 1) * 64]
                wqk_c[D_MODEL, cols + 128:cols + 192] = b_k[hA * 64:(hA + 1) * 64]
                wqk_c[D_MODEL, cols + 192:cols + 256] = b_k[hB * 64:(hB + 1) * 64]
        wv_c = np.ascontiguousarray(
            w_v[8 * g * 64:(8 * g + 8) * 64].T).astype(np.float16)
        wp_c = np.ascontiguousarray(
            w_proj.T[g * 512:(g + 1) * 512, :]).astype(np.float16)
        in_maps.append({
            "xT": xT_c, "wqk": wqk_c, "wv": wv_c, "wp": wp_c,
            "tri2": tri2, "ident": ident,
        })

    results = run(in_maps)

    out = np.empty((B, T, D_MODEL), dtype=np.float32)
    for b in range(B):
        out[b] = (results[2 * b]["out"].astype(np.float32)
                  + results[2 * b + 1]["out"].astype(np.float32))

    # exact host-side bias folds (v-bias rides softmax row-sums == 1;
    # proj bias is additive)
    if np.any(b_v):
        out += (b_v @ w_proj.T)[None, None, :]
    if np.any(b_proj):
        out += b_proj[None, None, :]
    return out


# revision 3
# speedup vs baseline: 1.0014x; 1.0014x over previous
"""Causal self-attention (B=4, T=2048, C=1024, H=16) on 8 TRN2 NeuronCores.

Sharding: core = 2*b + g (b = batch 0..3, g = head-group 0..1). Each core
computes qkv + attention for its batch and its 8 heads, then a PARTIAL
out-projection (contraction over its 512 y-columns) across the FULL output;
the host sums the two partials per batch. No collectives.

Key structure (all f16 operands, f32 PSUM accumulation):
- x / weights DMA'd once, SBUF-resident; ~55 DMAs total.
- QK^T: stationary k-tile [64, 128], moving q [64, 512] -> scores [kpos, q].
- exp on Act engine (no max-subtraction; logits are O(1) after 0.125 scale),
  f16 probabilities; causal mask = one 128-wide multiply per diagonal tile.
- PV streams V: stationary p-tile [128, 128], moving v|1 [128, 65]
  -> y [q, dh] at 65 cycles per score-tile (vs 512 the other orientation),
  with the softmax denominator riding the ones column.
- normalize: per-partition reciprocal + tensor_scalar multiply (q is the
  partition dim), then PE-transpose into y^T [c, q] for the out-projection.
- c-major software pipeline: scores(item) | pv(prev) | transposes(prevprev)
  interleaved with qkv-proj / out-proj filler to keep PE busy.
"""
import numpy as np

D_MODEL = 1024
N_HEAD = 16
D_HEAD = 64
B = 4
T = 2048
N_CORES = 8
P = 128
PAIRS = 4          # head pairs per core
KT = D_MODEL // P  # 8 contraction tiles
NQ = 4             # q-chunks of 512
QC = 512           # q chunk width

_RUNNER_CACHE = {}


def _build(has_qk_bias: bool):
    from concourse import bacc
    import concourse.mybir as mybir
    from concourse.tile import TileContext
    from concourse.bass import ts

    f32 = mybir.dt.float32
    f16 = mybir.dt.float16
    KD = D_MODEL + (1 if has_qk_bias else 0)

    nc = bacc.Bacc("TRN2", target_bir_lowering=False, debug=False,
                   num_devices=N_CORES)
    xT = nc.dram_tensor("xT", [KD, T], f16, kind="ExternalInput")
    wqk = nc.dram_tensor("wqk", [KD, 1024], f16, kind="ExternalInput")
    wv = nc.dram_tensor("wv", [D_MODEL, 512], f16, kind="ExternalInput")
    wp = nc.dram_tensor("wp", [512, 1024], f16, kind="ExternalInput")
    tri2 = nc.dram_tensor("tri2", [P, 2 * P], f16, kind="ExternalInput")
    ident = nc.dram_tensor("ident", [P, P], f16, kind="ExternalInput")
    out = nc.dram_tensor("out", [T, 1024], f16, kind="ExternalOutput")

    with TileContext(nc) as tc:
        with (
            tc.tile_pool(name="const", bufs=1) as cp,
            tc.tile_pool(name="qk_res", bufs=1) as qk_res,
            tc.tile_pool(name="v_res", bufs=1) as v_res,
            tc.tile_pool(name="yf_res", bufs=1) as yf_res,
            tc.tile_pool(name="ex", bufs=24) as ex_pool,
            tc.tile_pool(name="ys", bufs=18) as ys_pool,
            tc.tile_pool(name="rc", bufs=18) as rc_pool,
            tc.tile_pool(name="ob", bufs=4) as o_pool,
            tc.tile_pool(name="stp", bufs=2, space="PSUM") as st_pool,
            tc.tile_pool(name="pjp", bufs=2, space="PSUM") as pj_pool,
            tc.tile_pool(name="yp", bufs=2, space="PSUM") as y_pool,
        ):
            # ---------------- constants / inputs ----------------
            x_sb = cp.tile([P, KT, T], f16, name="x_sb")
            wqk_sb = cp.tile([P, KT, 1024], f16, name="wqk_sb")
            wv_sb = cp.tile([P, KT, 512], f16, name="wv_sb")
            wp_sb = cp.tile([P, 4, 1024], f16, name="wp_sb")
            tri_sb = cp.tile([P, 2, P], f16, name="tri_sb")
            id_sb = cp.tile([P, P], f16, name="id_sb")
            # DMA order tuned so vproj can start after wv + x-chunk0 and
            # qkproj(0) is never starved: id, wv, x0, wqk, x1..x3, wp, tri.
            nc.sync.dma_start(out=id_sb, in_=ident[:])
            nc.sync.dma_start(
                out=wv_sb, in_=wv[:].rearrange("(k p) c -> p k c", p=P))
            nc.sync.dma_start(
                out=x_sb[:, :, ts(0, QC)],
                in_=xT[0:D_MODEL, ts(0, QC)].rearrange("(k p) t -> p k t",
                                                       p=P))
            nc.sync.dma_start(
                out=wqk_sb,
                in_=wqk[0:D_MODEL, :].rearrange("(k p) c -> p k c", p=P))
            for nn in range(1, NQ):
                nc.sync.dma_start(
                    out=x_sb[:, :, ts(nn, QC)],
                    in_=xT[0:D_MODEL, ts(nn, QC)].rearrange("(k p) t -> p k t",
                                                            p=P))
            nc.sync.dma_start(
                out=wp_sb, in_=wp[:].rearrange("(k p) c -> p k c", p=P))
            nc.sync.dma_start(out=tri_sb, in_=tri2[:].rearrange(
                "p (j c) -> p j c", j=2))
            if has_qk_bias:
                xrow = cp.tile([1, T], f16, name="xrow")
                nc.sync.dma_start(out=xrow, in_=xT[D_MODEL:D_MODEL + 1, :])
                wrow = cp.tile([1, 1024], f16, name="wrow")
                nc.sync.dma_start(out=wrow, in_=wqk[D_MODEL:D_MODEL + 1, :])

            qT = [qk_res.tile([P, T], f16, name=f"qT{p}") for p in range(PAIRS)]
            kT = [qk_res.tile([P, T], f16, name=f"kT{p}") for p in range(PAIRS)]
            v_sb = [v_res.tile([P, 8, 65], f16, name=f"v{t}")
                    for t in range(T // P)]
            yf = [yf_res.tile([P, T], f16, name=f"yf{p}") for p in range(PAIRS)]

            # ---------------- emit helpers ----------------
            def vproj(tt):
                """V projection for t-tile tt: v_sb[tt] <- x_tile @ wv."""
                ps = pj_pool.tile([P, 512], f32, name="vps", tag="pj")
                n, tl = divmod(tt, 4)
                for k in range(KT):
                    nc.tensor.matmul(ps[:], x_sb[:, k, ts(n, QC)][:, ts(tl, P)],
                                     wv_sb[:, k, :],
                                     start=(k == 0), stop=(k == KT - 1))
                nc.vector.memset(v_sb[tt][:, :, 64:65], 1.0)
                src = ps.rearrange("p (h c) -> p h c", c=64)
                nc.vector.tensor_copy(out=v_sb[tt][:, :, 0:64], in_=src[:])

            def qkproj_nm(p, n, m):
                dest = qT[p] if m == 0 else kT[p]
                ps = pj_pool.tile([P, 512], f32, name="qkps", tag="pj")
                for k in range(KT):
                    nc.tensor.matmul(
                        ps[:],
                        wqk_sb[:, k, p * 256 + m * P:p * 256 + (m + 1) * P],
                        x_sb[:, k, ts(n, QC)],
                        start=(k == 0),
                        stop=(k == KT - 1) and not has_qk_bias)
                if has_qk_bias:
                    nc.tensor.matmul(
                        ps[:],
                        wrow[:, p * 256 + m * P:p * 256 + (m + 1) * P],
                        xrow[:, ts(n, QC)],
                        start=False, stop=True)
                nc.vector.tensor_copy(out=dest[:, ts(n, QC)], in_=ps[:])

            def qkproj(p):
                """q/k projection for pair p -> qT[p], kT[p] (f16)."""
                for n in range(NQ):
                    for m in (0, 1):
                        qkproj_nm(p, n, m)

            def scores(p, c, filler=(), hs=(0, 1)):
                """QK^T + exp + causal mask for (pair p, q-chunk c).
                Emits one unit from `filler` (a list of closures) after each
                k-tile group so PE has independent queued work while the
                2-deep st ring throttles QK^T to the Act exp cadence.
                Returns ex-tile refs for pv()."""
                filler = list(filler)

                def fill(n=1):
                    for _ in range(n):
                        if filler:
                            filler.pop(0)()

                exs = [[None] * (2 * c) for _ in range(2)]  # [h][g]
                exd = [None] * 4                            # diagonal, h-stacked
                for h in hs:
                    pb = h * 64
                    for g in range(2 * c):
                        st = st_pool.tile([P, 2, QC], f32, name="st", tag="st")
                        for j, kt in enumerate((2 * g, 2 * g + 1)):
                            nc.tensor.matmul(st[:, j, :],
                                             kT[p][pb:pb + 64, ts(kt, P)],
                                             qT[p][pb:pb + 64, ts(c, QC)],
                                             start=True, stop=True)
                        ex = ex_pool.tile([P, 2, QC], f16, name="ex", tag="ex")
                        nc.scalar.activation(
                            ex[:], st[:], mybir.ActivationFunctionType.Exp,
                            scale=0.125)
                        exs[h][g] = ex
                        fill()
                # diagonal tiles, h-stacked; d1 and d3 share one st tile
                # ([0:384] and [384:512]) so their exp is a single
                # instruction. start/stop flags per psum zero-region.
                for ds in ((0,), (1, 3), (2,)):
                    kt0 = 4 * c + ds[0]
                    st = st_pool.tile([P, 2, QC], f32, name="std", tag="st")
                    off = 0
                    offs = []
                    for d in ds:
                        offs.append(off)
                        w = QC - P * d
                        for h in hs:
                            pb = h * 64
                            nc.tensor.matmul(
                                st[:, h, off:off + w],
                                kT[p][pb:pb + 64, ts(4 * c + d, P)],
                                qT[p][pb:pb + 64,
                                      c * QC + P * d:(c + 1) * QC],
                                start=(d == ds[0]), stop=(d == ds[-1]))
                        off += w
                    ex = ex_pool.tile([P, 2, QC], f16, name="exd", tag="ex")
                    if len(hs) == 2:
                        nc.scalar.activation(
                            ex[:, :, 0:off], st[:, :, 0:off],
                            mybir.ActivationFunctionType.Exp, scale=0.125)
                    else:
                        nc.scalar.activation(
                            ex[:, hs[0], 0:off], st[:, hs[0], 0:off],
                            mybir.ActivationFunctionType.Exp, scale=0.125)
                    # causal mask on each 128-wide diagonal block; on Pool so
                    # DVE (yf copies feeding outproj) never queues behind an
                    # Act-dependent op
                    for d, off in zip(ds, offs):
                        if len(hs) == 2:
                            nc.gpsimd.tensor_mul(ex[:, :, off:off + P],
                                                 ex[:, :, off:off + P],
                                                 tri_sb[:])
                        else:
                            nc.gpsimd.tensor_mul(
                                ex[:, hs[0], off:off + P],
                                ex[:, hs[0], off:off + P], tri_sb[:, 0, :])
                        exd[d] = (ex, off)
                    fill()
                fill(len(filler))
                return exs, exd

            def pv_norm_units(p, c, exs, exd, ys_all, hs=(0, 1)):
                """PV + normalize closures for (p, c); executing all of them
                (in order) fills ys_all[h][qt_rel] with f16 tiles."""
                units = []
                state = {}
                CH = 8
                for h in hs:
                    lh = 2 * p + h
                    mms = []
                    for kt in range(4 * c + 4):
                        for qt_rel in range(4):
                            qlo = qt_rel * P
                            if kt < 4 * c:
                                exsl = exs[h][kt // 2][:, kt % 2, qlo:qlo + P]
                            else:
                                d = kt - 4 * c
                                so = P * d
                                if qlo < so:
                                    continue
                                exd_t, exd_o = exd[d]
                                exsl = exd_t[:, h,
                                             exd_o + qlo - so:
                                             exd_o + qlo - so + P]
                            mms.append((qt_rel, exsl, v_sb[kt][:, lh, :]))
                    nch = (len(mms) + CH - 1) // CH

                    def pvu(h=h, ci=None, mms=mms, nch=nch):
                        if ci == 0:
                            state[h] = y_pool.tile([P, 4, 65], f32,
                                                   name="y_ps", tag="y")
                        y_ps = state[h]
                        for i in range(ci * CH, min((ci + 1) * CH, len(mms))):
                            qt_rel, exsl, vsl = mms[i]
                            nc.tensor.matmul(y_ps[:, qt_rel, :], exsl, vsl,
                                             start=(i == 0),
                                             stop=(i == len(mms) - 1))
                        if ci == nch - 1:
                            rc = rc_pool.tile([P, 4], f32, name="rc", tag="rc")
                            nc.vector.reciprocal(out=rc[:],
                                                 in_=y_ps[:, :, 64:65])
                            state[(h, 'rc')] = rc
                    for ci in range(nch):
                        units.append(lambda h=h, ci=ci, f=pvu: f(ci=ci))

                def normu(qts):
                    for qt_rel in qts:
                        for h in hs:
                            yst = ys_pool.tile([P, 64], f16, name="ys",
                                               tag="ys")
                            nc.vector.tensor_scalar(
                                out=yst[:], in0=state[h][:, qt_rel, 0:64],
                                scalar1=state[(h, 'rc')][:, qt_rel:qt_rel + 1],
                                scalar2=None, op0=mybir.AluOpType.mult)
                            ys_all[h][qt_rel] = yst
                units.append(lambda: normu((0, 1)))
                units.append(lambda: normu((2, 3)))
                return units

            def trans_copy_units(p, c, ys_all):
                """Transpose ys -> y^T psum, assemble yf[p] (f16)."""
                def u(qt_rel):
                    qt = 4 * c + qt_rel
                    yt = y_pool.tile([P, P], f16, name="yt", tag="y")
                    nc.tensor.transpose(yt[0:64, :], ys_all[0][qt_rel], id_sb)
                    nc.tensor.transpose(yt[64:128, :], ys_all[1][qt_rel],
                                        id_sb)
                    nc.vector.tensor_copy(out=yf[p][:, ts(qt, P)], in_=yt[:])
                return [lambda qt_rel=q: u(qt_rel) for q in range(4)]

            def outproj_tt(tt):
                """Partial out-projection for one t-tile (needs all yf)."""
                for oc in (0, 1):
                    ps = pj_pool.tile([P, 512], f32, name="ops", tag="pj")
                    for ct in range(4):
                        nc.tensor.matmul(
                            ps[:], yf[ct][:, ts(tt, P)],
                            wp_sb[:, ct, ts(oc, 512)],
                            start=(ct == 0), stop=(ct == 3))
                    ot = o_pool.tile([P, 512], f16, name="ot", tag="ot")
                    nc.vector.tensor_copy(out=ot[:], in_=ps[:])
                    nc.sync.dma_start(out=out[ts(tt, P), ts(oc, 512)],
                                      in_=ot[:])

            def outproj(c):
                for tt in range(4 * c, 4 * c + 4):
                    outproj_tt(tt)

            # ---------------- schedule (c-major, 2-deep pipeline) ---------
            # PE warmup during the input-DMA head: ramps the p-state clock
            # (0.65 -> 2.4 GHz needs 3us of continuous busy) on zero-matmuls
            # with no DMA dependency.
            zw = cp.tile([P, QC], f16, name="zw")
            nc.vector.memset(zw, 0.0)
            warm = pj_pool.tile([P, QC], f32, name="warm", tag="pj")
            for i in range(14):
                nc.tensor.matmul(warm[:], zw[:, 0:P], zw[:],
                                 start=True, stop=True)

            def qk_units(p):
                return [(lambda n=n, m=m: qkproj_nm(p, n, m))
                        for n in range(NQ) for m in (0, 1)]

            def vp_unit(tt):
                return lambda: vproj(tt)

            def op_unit(tt):
                return lambda: outproj_tt(tt)

            # c-major item order; filler routed so the Act-bound c2/c3
            # stretches get the deferred out-projections as PE work:
            #   c0: qkproj(p+1) + vproj(4..7)
            #   c1 (p3): vproj(8..11)
            #   c2: vproj(12..15) + outproj(0), one of each per pair
            #   c3: outproj(1) and outproj(2), two per pair
            #   tail: outproj(3)
            for tt in range(4):
                vproj(tt)
            qkproj(0)
            post_map = {}
            for p in range(3):
                post_map[p] = qk_units(p + 1) + [vp_unit(4 + p)]
            post_map[3] = [vp_unit(7)]
            post_map[7] = [vp_unit(t) for t in range(8, 12)]
            # outproj(c) units may only appear after trans_copy((3, c)),
            # which trails item (3, c) by two pipeline slots.
            post_map[8] = [vp_unit(12)]
            post_map[9] = [vp_unit(13), op_unit(0)]
            post_map[10] = [op_unit(1), op_unit(2)]
            post_map[11] = [vp_unit(14), vp_unit(15), op_unit(3)]
            post_map[12] = [op_unit(4)]
            post_map[13] = [op_unit(5), op_unit(8)]
            post_map[14] = [op_unit(6), op_unit(9)]
            prev = prevprev = None     # (p, c, exs, exd) / (p, c, ys_all)
            items = [(p, c) for c in range(NQ) for p in range(PAIRS)][:-1]
            for i, (p, c) in enumerate(items):
                filler = []
                if prevprev is not None:
                    qp, qc2, qys = prevprev
                    filler += trans_copy_units(qp, qc2, qys)
                    prevprev = None
                if prev is not None:
                    pp, pc, pexs, pexd = prev
                    ys_all = [[None] * 4 for _ in range(2)]
                    filler += pv_norm_units(pp, pc, pexs, pexd, ys_all)
                    prevprev = (pp, pc, ys_all)
                filler += post_map.get(i, [])
                sc = scores(p, c, filler)
                prev = (p, c, sc[0], sc[1])
            # ----- tail: h-split the last item (3, 3) so the terminal
            # Act burst is halved, with the final out-projections as filler --
            fillerA = []
            qp, qc2, qys = prevprev
            fillerA += trans_copy_units(qp, qc2, qys)
            pp, pc, pexs, pexd = prev
            ys23 = [[None] * 4 for _ in range(2)]
            fillerA += pv_norm_units(pp, pc, pexs, pexd, ys23)
            fillerA.append(op_unit(7))
            scA = scores(3, 3, fillerA, hs=(0,))
            ys33 = [[None] * 4 for _ in range(2)]
            fillerB = trans_copy_units(pp, pc, ys23)
            fillerB += pv_norm_units(3, 3, scA[0], scA[1], ys33, hs=(0,))
            fillerB.append(op_unit(10))
            scB = scores(3, 3, fillerB, hs=(1,))
            outproj_tt(11)   # runs while the final exp burst drains
            for u in pv_norm_units(3, 3, scB[0], scB[1], ys33, hs=(1,)):
                u()
            tail_trans = trans_copy_units(3, 3, ys33)
            for qt_rel in range(4):
                tail_trans[qt_rel]()
                outproj_tt(12 + qt_rel)

    nc.compile()
    return nc


def _make_runner(nc):
    """Reusable 8-core SPMD runner (jit built once)."""
    import jax
    from jax.sharding import Mesh, PartitionSpec
    from jax.experimental.shard_map import shard_map
    from concourse import bass2jax
    import concourse.mybir as mybir

    bass2jax.install_neuronx_cc_hook()
    partition_name = (nc.partition_id_tensor.name
                      if nc.partition_id_tensor else None)
    in_names, out_names, out_avals, zero_outs = [], [], [], []
    for alloc in nc.m.functions[0].allocations:
        if not isinstance(alloc, mybir.MemoryLocationSet):
            continue
        name = alloc.memorylocations[0].name
        if alloc.kind == "ExternalInput":
            if name != partition_name:
                in_names.append(name)
        elif alloc.kind == "ExternalOutput":
            shape = tuple(alloc.tensor_shape)
            dtype = mybir.dt.np(alloc.dtype)
            out_names.append(name)
            out_avals.append(jax.core.ShapedArray(shape, dtype))
            zero_outs.append(np.zeros(shape, dtype))
    n_params = len(in_names)
    n_outs = len(out_avals)
    all_in = list(in_names) + list(out_names)
    if partition_name is not None:
        all_in.append(partition_name)

    def _body(*args):
        operands = list(args)
        if partition_name is not None:
            operands.append(bass2jax.partition_id_tensor())
        outs = bass2jax._bass_exec_p.bind(
            *operands,
            out_avals=tuple(out_avals),
            in_names=tuple(all_in),
            out_names=tuple(out_names),
            lowering_input_output_aliases=(),
            sim_require_finite=True,
            sim_require_nnan=True,
            nc=nc,
        )
        return tuple(outs)

    devices = jax.devices()[:N_CORES]
    mesh = Mesh(np.asarray(devices), ("core",))
    in_specs = (PartitionSpec("core"),) * (n_params + n_outs)
    out_specs = (PartitionSpec("core"),) * n_outs
    donate = tuple(range(n_params, n_params + n_outs))
    sharded = jax.jit(
        shard_map(_body, mesh=mesh, in_specs=in_specs, out_specs=out_specs,
                  check_rep=False),
        donate_argnums=donate, keep_unused=True)

    def run(in_maps):
        per_core = [[np.asarray(m[k]) for k in in_names] for m in in_maps]
        concat_in = [
            np.concatenate([per_core[c][i] for c in range(N_CORES)], axis=0)
            for i in range(n_params)]
        concat_zeros = [
            np.zeros((N_CORES * z.shape[0], *z.shape[1:]), z.dtype)
            for z in zero_outs]
        outs = sharded(*concat_in, *concat_zeros)
        jax.block_until_ready(outs)
        return [
            {name: np.asarray(outs[i]).reshape(N_CORES, *out_avals[i].shape)[c]
             for i, name in enumerate(out_names)}
            for c in range(N_CORES)]

    return run


def kernel(x, w_qkv, b_qkv, w_proj, b_proj):
    x = np.asarray(x, dtype=np.float32)
    w_qkv = np.asarray(w_qkv, dtype=np.float32)
    b_qkv = np.asarray(b_qkv, dtype=np.float32)
    w_proj = np.asarray(w_proj, dtype=np.float32)
    b_proj = np.asarray(b_proj, dtype=np.float32)

    w_q, w_k, w_v = w_qkv[0:1024], w_qkv[1024:2048], w_qkv[2048:3072]
    b_q, b_k, b_v = b_qkv[0:1024], b_qkv[1024:2048], b_qkv[2048:3072]
    has_qk_bias = bool(np.any(b_q) or np.any(b_k))

    key = ("runner", has_qk_bias)
    if key not in _RUNNER_CACHE:
        nc = _build(has_qk_bias)
        _RUNNER_CACHE[key] = _make_runner(nc)
    run = _RUNNER_CACHE[key]

    # causal block mask: tri2[k, j*128 + q] = 1.0 iff k <= q
    kk = np.arange(P)[:, None]
    qq = np.arange(P)[None, :]
    blk = (kk <= qq).astype(np.float16)
    tri2 = np.concatenate([blk, blk], axis=1)
    ident = np.eye(P, dtype=np.float16)

    in_maps = []
    for core in range(N_CORES):
        b, g = divmod(core, 2)
        xT_c = np.ascontiguousarray(x[b].T).astype(np.float16)
        if has_qk_bias:
            xT_c = np.concatenate([xT_c, np.ones((1, T), np.float16)], axis=0)
        wqk_c = np.empty((D_MODEL + (1 if has_qk_bias else 0), 1024),
                         np.float16)
        for p in range(PAIRS):
            hA = 8 * g + 2 * p
            hB = hA + 1
            cols = p * 256
            wqk_c[:D_MODEL, cols + 0:cols + 64] = w_q[hA * 64:(hA + 1) * 64].T
            wqk_c[:D_MODEL, cols + 64:cols + 128] = w_q[hB * 64:(hB + 1) * 64].T
            wqk_c[:D_MODEL, cols + 128:cols + 192] = w_k[hA * 64:(hA + 1) * 64].T
            wqk_c[:D_MODEL, cols + 192:cols + 256] = w_k[hB * 64:(hB + 1) * 64].T
            if has_qk_bias:
                wqk_c[D_MODEL, cols + 0:cols + 64] = b_q[hA * 64:(hA + 1) * 64]
                wqk_c[D_MODEL, cols + 64:cols + 128] = b_q[hB * 64:(hB + 1) * 64]
                wqk_c[D_MODEL, cols + 128:cols + 192] = b_k[hA * 64:(hA + 1) * 64]
                wqk_c[D_MODEL, cols + 192:cols + 256] = b_k[hB * 64:(hB + 1) * 64]
        wv_c = np.ascontiguousarray(
            w_v[8 * g * 64:(8 * g + 8) * 64].T).astype(np.float16)
        wp_c = np.ascontiguousarray(
            w_proj.T[g * 512:(g + 1) * 512, :]).astype(np.float16)
        in_maps.append({
            "xT": xT_c, "wqk": wqk_c, "wv": wv_c, "wp": wp_c,
            "tri2": tri2, "ident": ident,
        })

    results = run(in_maps)

    out = np.empty((B, T, D_MODEL), dtype=np.float32)
    for b in range(B):
        out[b] = (results[2 * b]["out"].astype(np.float32)
                  + results[2 * b + 1]["out"].astype(np.float32))

    # exact host-side bias folds (v-bias rides softmax row-sums == 1;
    # proj bias is additive)
    if np.any(b_v):
        out += (b_v @ w_proj.T)[None, None, :]
    if np.any(b_proj):
        out += b_proj[None, None, :]
    return out


# revision 4
# speedup vs baseline: 1.0026x; 1.0012x over previous
"""Causal self-attention (B=4, T=2048, C=1024, H=16) on 8 TRN2 NeuronCores.

Sharding: core = 2*b + g (b = batch 0..3, g = head-group 0..1). Each core
computes qkv + attention for its batch and its 8 heads, then a PARTIAL
out-projection (contraction over its 512 y-columns) across the FULL output;
the host sums the two partials per batch. No collectives.

Key structure (all f16 operands, f32 PSUM accumulation):
- x / weights DMA'd once, SBUF-resident; ~55 DMAs total.
- QK^T: stationary k-tile [64, 128], moving q [64, 512] -> scores [kpos, q].
- exp on Act engine (no max-subtraction; logits are O(1) after 0.125 scale),
  f16 probabilities; causal mask = one 128-wide multiply per diagonal tile.
- PV streams V: stationary p-tile [128, 128], moving v|1 [128, 65]
  -> y [q, dh] at 65 cycles per score-tile (vs 512 the other orientation),
  with the softmax denominator riding the ones column.
- normalize: per-partition reciprocal + tensor_scalar multiply (q is the
  partition dim), then PE-transpose into y^T [c, q] for the out-projection.
- c-major software pipeline: scores(item) | pv(prev) | transposes(prevprev)
  interleaved with qkv-proj / out-proj filler to keep PE busy.
"""
import numpy as np

D_MODEL = 1024
N_HEAD = 16
D_HEAD = 64
B = 4
T = 2048
N_CORES = 8
P = 128
PAIRS = 4          # head pairs per core
KT = D_MODEL // P  # 8 contraction tiles
NQ = 4             # q-chunks of 512
QC = 512           # q chunk width

_RUNNER_CACHE = {}


def _build(has_qk_bias: bool):
    from concourse import bacc
    import concourse.mybir as mybir
    from concourse.tile import TileContext
    from concourse.bass import ts

    f32 = mybir.dt.float32
    f16 = mybir.dt.float16
    KD = D_MODEL + (1 if has_qk_bias else 0)

    nc = bacc.Bacc("TRN2", target_bir_lowering=False, debug=False,
                   num_devices=N_CORES)
    xT = nc.dram_tensor("xT", [KD, T], f16, kind="ExternalInput")
    wqk = nc.dram_tensor("wqk", [KD, 1024], f16, kind="ExternalInput")
    wv = nc.dram_tensor("wv", [D_MODEL, 512], f16, kind="ExternalInput")
    wp = nc.dram_tensor("wp", [512, 1024], f16, kind="ExternalInput")
    tri2 = nc.dram_tensor("tri2", [P, 2 * P], f16, kind="ExternalInput")
    ident = nc.dram_tensor("ident", [P, P], f16, kind="ExternalInput")
    out = nc.dram_tensor("out", [T, 1024], f16, kind="ExternalOutput")

    with TileContext(nc) as tc:
        with (
            tc.tile_pool(name="const", bufs=1) as cp,
            tc.tile_pool(name="qk_res", bufs=1) as qk_res,
            tc.tile_pool(name="v_res", bufs=1) as v_res,
            tc.tile_pool(name="yf_res", bufs=1) as yf_res,
            tc.tile_pool(name="ex", bufs=20) as ex_pool,
            tc.tile_pool(name="ys", bufs=18) as ys_pool,
            tc.tile_pool(name="rc", bufs=18) as rc_pool,
            tc.tile_pool(name="ob", bufs=4) as o_pool,
            tc.tile_pool(name="stp", bufs=2, space="PSUM") as st_pool,
            tc.tile_pool(name="pjp", bufs=2, space="PSUM") as pj_pool,
            tc.tile_pool(name="yp", bufs=2, space="PSUM") as y_pool,
        ):
            # ---------------- constants / inputs ----------------
            x_sb = cp.tile([P, KT, T], f16, name="x_sb")
            wqk_sb = cp.tile([P, KT, 1024], f16, name="wqk_sb")
            wv_sb = cp.tile([P, KT, 512], f16, name="wv_sb")
            wp_sb = cp.tile([P, 4, 1024], f16, name="wp_sb")
            tri_sb = cp.tile([P, 2, P], f16, name="tri_sb")
            id_sb = cp.tile([P, P], f16, name="id_sb")
            # DMA order tuned so vproj can start after wv + x-chunk0 and
            # qkproj(0) is never starved: id, wv, x0, wqk, x1..x3, wp, tri.
            nc.sync.dma_start(out=id_sb, in_=ident[:])
            nc.sync.dma_start(
                out=wv_sb, in_=wv[:].rearrange("(k p) c -> p k c", p=P))
            nc.sync.dma_start(
                out=x_sb[:, :, ts(0, QC)],
                in_=xT[0:D_MODEL, ts(0, QC)].rearrange("(k p) t -> p k t",
                                                       p=P))
            nc.sync.dma_start(
                out=wqk_sb,
                in_=wqk[0:D_MODEL, :].rearrange("(k p) c -> p k c", p=P))
            for nn in range(1, NQ):
                nc.sync.dma_start(
                    out=x_sb[:, :, ts(nn, QC)],
                    in_=xT[0:D_MODEL, ts(nn, QC)].rearrange("(k p) t -> p k t",
                                                            p=P))
            nc.sync.dma_start(
                out=wp_sb, in_=wp[:].rearrange("(k p) c -> p k c", p=P))
            nc.sync.dma_start(out=tri_sb, in_=tri2[:].rearrange(
                "p (j c) -> p j c", j=2))
            if has_qk_bias:
                xrow = cp.tile([1, T], f16, name="xrow")
                nc.sync.dma_start(out=xrow, in_=xT[D_MODEL:D_MODEL + 1, :])
                wrow = cp.tile([1, 1024], f16, name="wrow")
                nc.sync.dma_start(out=wrow, in_=wqk[D_MODEL:D_MODEL + 1, :])

            qT = [qk_res.tile([P, T], f16, name=f"qT{p}") for p in range(PAIRS)]
            kT = [qk_res.tile([P, T], f16, name=f"kT{p}") for p in range(PAIRS)]
            v_sb = [v_res.tile([P, 8, 65], f16, name=f"v{t}")
                    for t in range(T // P)]
            yf = [yf_res.tile([P, T], f16, name=f"yf{p}") for p in range(PAIRS)]

            # ---------------- emit helpers ----------------
            def vproj(tt):
                """V projection for t-tile tt: v_sb[tt] <- x_tile @ wv."""
                ps = pj_pool.tile([P, 512], f32, name="vps", tag="pj")
                n, tl = divmod(tt, 4)
                for k in range(KT):
                    nc.tensor.matmul(ps[:], x_sb[:, k, ts(n, QC)][:, ts(tl, P)],
                                     wv_sb[:, k, :],
                                     start=(k == 0), stop=(k == KT - 1))
                nc.vector.memset(v_sb[tt][:, :, 64:65], 1.0)
                src = ps.rearrange("p (h c) -> p h c", c=64)
                nc.vector.tensor_copy(out=v_sb[tt][:, :, 0:64], in_=src[:])

            def qkproj_nm(p, n, m):
                dest = qT[p] if m == 0 else kT[p]
                ps = pj_pool.tile([P, 512], f32, name="qkps", tag="pj")
                for k in range(KT):
                    nc.tensor.matmul(
                        ps[:],
                        wqk_sb[:, k, p * 256 + m * P:p * 256 + (m + 1) * P],
                        x_sb[:, k, ts(n, QC)],
                        start=(k == 0),
                        stop=(k == KT - 1) and not has_qk_bias)
                if has_qk_bias:
                    nc.tensor.matmul(
                        ps[:],
                        wrow[:, p * 256 + m * P:p * 256 + (m + 1) * P],
                        xrow[:, ts(n, QC)],
                        start=False, stop=True)
                nc.vector.tensor_copy(out=dest[:, ts(n, QC)], in_=ps[:])

            def qkproj(p):
                """q/k projection for pair p -> qT[p], kT[p] (f16)."""
                for n in range(NQ):
                    for m in (0, 1):
                        qkproj_nm(p, n, m)

            def scores(p, c, filler=(), hs=(0, 1)):
                """QK^T + exp + causal mask for (pair p, q-chunk c).
                Emits one unit from `filler` (a list of closures) after each
                k-tile group so PE has independent queued work while the
                2-deep st ring throttles QK^T to the Act exp cadence.
                Returns ex-tile refs for pv()."""
                filler = list(filler)

                def fill(n=1):
                    for _ in range(n):
                        if filler:
                            filler.pop(0)()

                exs = [[None] * (2 * c) for _ in range(2)]  # [h][g]
                exd = [None] * 4                            # diagonal, h-stacked
                for h in hs:
                    pb = h * 64
                    for g in range(2 * c):
                        st = st_pool.tile([P, 2, QC], f32, name="st", tag="st")
                        for j, kt in enumerate((2 * g, 2 * g + 1)):
                            nc.tensor.matmul(st[:, j, :],
                                             kT[p][pb:pb + 64, ts(kt, P)],
                                             qT[p][pb:pb + 64, ts(c, QC)],
                                             start=True, stop=True)
                        ex = ex_pool.tile([P, 2, QC], f16, name="ex", tag="ex")
                        nc.scalar.activation(
                            ex[:], st[:], mybir.ActivationFunctionType.Exp,
                            scale=0.125)
                        exs[h][g] = ex
                        fill()
                # diagonal tiles, h-stacked; d1 and d3 share one st tile
                # ([0:384] and [384:512]) so their exp is a single
                # instruction. start/stop flags per psum zero-region.
                for ds in ((0,), (1, 3), (2,)):
                    kt0 = 4 * c + ds[0]
                    st = st_pool.tile([P, 2, QC], f32, name="std", tag="st")
                    off = 0
                    offs = []
                    for d in ds:
                        offs.append(off)
                        w = QC - P * d
                        for h in hs:
                            pb = h * 64
                            nc.tensor.matmul(
                                st[:, h, off:off + w],
                                kT[p][pb:pb + 64, ts(4 * c + d, P)],
                                qT[p][pb:pb + 64,
                                      c * QC + P * d:(c + 1) * QC],
                                start=(d == ds[0]), stop=(d == ds[-1]))
                        off += w
                    ex = ex_pool.tile([P, 2, QC], f16, name="exd", tag="ex")
                    if len(hs) == 2:
                        nc.scalar.activation(
                            ex[:, :, 0:off], st[:, :, 0:off],
                            mybir.ActivationFunctionType.Exp, scale=0.125)
                    else:
                        nc.scalar.activation(
                            ex[:, hs[0], 0:off], st[:, hs[0], 0:off],
                            mybir.ActivationFunctionType.Exp, scale=0.125)
                    # causal mask on each 128-wide diagonal block; on Pool so
                    # DVE (yf copies feeding outproj) never queues behind an
                    # Act-dependent op
                    for d, off in zip(ds, offs):
                        if len(hs) == 2:
                            nc.gpsimd.tensor_mul(ex[:, :, off:off + P],
                                                 ex[:, :, off:off + P],
                                                 tri_sb[:])
                        else:
                            nc.gpsimd.tensor_mul(
                                ex[:, hs[0], off:off + P],
                                ex[:, hs[0], off:off + P], tri_sb[:, 0, :])
                        exd[d] = (ex, off)
                    fill()
                fill(len(filler))
                return exs, exd

            def pv_norm_units(p, c, exs, exd, ys_all, hs=(0, 1)):
                """PV + normalize closures for (p, c); executing all of them
                (in order) fills ys_all[h][qt_rel] with f16 tiles."""
                units = []
                state = {}
                CH = 8
                for h in hs:
                    lh = 2 * p + h
                    mms = []
                    for kt in range(4 * c + 4):
                        for qt_rel in range(4):
                            qlo = qt_rel * P
                            if kt < 4 * c:
                                exsl = exs[h][kt // 2][:, kt % 2, qlo:qlo + P]
                            else:
                                d = kt - 4 * c
                                so = P * d
                                if qlo < so:
                                    continue
                                exd_t, exd_o = exd[d]
                                exsl = exd_t[:, h,
                                             exd_o + qlo - so:
                                             exd_o + qlo - so + P]
                            mms.append((qt_rel, exsl, v_sb[kt][:, lh, :]))
                    nch = (len(mms) + CH - 1) // CH

                    def pvu(h=h, ci=None, mms=mms, nch=nch):
                        if ci == 0:
                            state[h] = y_pool.tile([P, 4, 65], f32,
                                                   name="y_ps", tag="y")
                        y_ps = state[h]
                        for i in range(ci * CH, min((ci + 1) * CH, len(mms))):
                            qt_rel, exsl, vsl = mms[i]
                            nc.tensor.matmul(y_ps[:, qt_rel, :], exsl, vsl,
                                             start=(i == 0),
                                             stop=(i == len(mms) - 1))
                        if ci == nch - 1:
                            rc = rc_pool.tile([P, 4], f32, name="rc", tag="rc")
                            nc.vector.reciprocal(out=rc[:],
                                                 in_=y_ps[:, :, 64:65])
                            state[(h, 'rc')] = rc
                    for ci in range(nch):
                        units.append(lambda h=h, ci=ci, f=pvu: f(ci=ci))

                def normu(qts):
                    for qt_rel in qts:
                        for h in hs:
                            yst = ys_pool.tile([P, 64], f16, name="ys",
                                               tag="ys")
                            nc.vector.tensor_scalar(
                                out=yst[:], in0=state[h][:, qt_rel, 0:64],
                                scalar1=state[(h, 'rc')][:, qt_rel:qt_rel + 1],
                                scalar2=None, op0=mybir.AluOpType.mult)
                            ys_all[h][qt_rel] = yst
                units.append(lambda: normu((0, 1)))
                units.append(lambda: normu((2, 3)))
                return units

            def trans_copy_units(p, c, ys_all):
                """Transpose ys -> y^T psum, assemble yf[p] (f16)."""
                def u(qt_rel):
                    qt = 4 * c + qt_rel
                    yt = y_pool.tile([P, P], f16, name="yt", tag="y")
                    nc.tensor.transpose(yt[0:64, :], ys_all[0][qt_rel], id_sb)
                    nc.tensor.transpose(yt[64:128, :], ys_all[1][qt_rel],
                                        id_sb)
                    nc.vector.tensor_copy(out=yf[p][:, ts(qt, P)], in_=yt[:])
                return [lambda qt_rel=q: u(qt_rel) for q in range(4)]

            def outproj_tt(tt):
                """Partial out-projection for one t-tile (needs all yf)."""
                for oc in (0, 1):
                    ps = pj_pool.tile([P, 512], f32, name="ops", tag="pj")
                    for ct in range(4):
                        nc.tensor.matmul(
                            ps[:], yf[ct][:, ts(tt, P)],
                            wp_sb[:, ct, ts(oc, 512)],
                            start=(ct == 0), stop=(ct == 3))
                    ot = o_pool.tile([P, 512], f16, name="ot", tag="ot")
                    nc.vector.tensor_copy(out=ot[:], in_=ps[:])
                    nc.sync.dma_start(out=out[ts(tt, P), ts(oc, 512)],
                                      in_=ot[:])

            def outproj(c):
                for tt in range(4 * c, 4 * c + 4):
                    outproj_tt(tt)

            # ---------------- schedule (c-major, 2-deep pipeline) ---------
            # PE warmup during the input-DMA head: ramps the p-state clock
            # (0.65 -> 2.4 GHz needs 3us of continuous busy) on zero-matmuls
            # with no DMA dependency.
            zw = cp.tile([P, QC], f16, name="zw")
            nc.vector.memset(zw, 0.0)
            warm = pj_pool.tile([P, QC], f32, name="warm", tag="pj")
            for i in range(14):
                nc.tensor.matmul(warm[:], zw[:, 0:P], zw[:],
                                 start=True, stop=True)

            def qk_units(p):
                return [(lambda n=n, m=m: qkproj_nm(p, n, m))
                        for n in range(NQ) for m in (0, 1)]

            def vp_unit(tt):
                return lambda: vproj(tt)

            def op_unit(tt):
                return lambda: outproj_tt(tt)

            # c-major item order; filler routed so the Act-bound c2/c3
            # stretches get the deferred out-projections as PE work:
            #   c0: qkproj(p+1) + vproj(4..7)
            #   c1 (p3): vproj(8..11)
            #   c2: vproj(12..15) + outproj(0), one of each per pair
            #   c3: outproj(1) and outproj(2), two per pair
            #   tail: outproj(3)
            for tt in range(4):
                vproj(tt)
            qkproj(0)
            post_map = {}
            for p in range(3):
                post_map[p] = qk_units(p + 1) + [vp_unit(4 + p)]
            post_map[3] = [vp_unit(7)]
            post_map[7] = [vp_unit(t) for t in range(8, 12)]
            # outproj(c) units may only appear after trans_copy((3, c)),
            # which trails item (3, c) by two pipeline slots.
            post_map[8] = [vp_unit(12)]
            post_map[9] = [vp_unit(13), op_unit(0)]
            post_map[10] = [op_unit(1), op_unit(2)]
            post_map[11] = [vp_unit(14), vp_unit(15), op_unit(3)]
            post_map[12] = [op_unit(4)]
            post_map[13] = [op_unit(5), op_unit(8)]
            post_map[14] = [op_unit(6), op_unit(9)]
            prev = prevprev = None     # (p, c, exs, exd) / (p, c, ys_all)
            items = [(p, c) for c in range(NQ) for p in range(PAIRS)][:-1]
            for i, (p, c) in enumerate(items):
                filler = []
                if prevprev is not None:
                    qp, qc2, qys = prevprev
                    filler += trans_copy_units(qp, qc2, qys)
                    prevprev = None
                if prev is not None:
                    pp, pc, pexs, pexd = prev
                    ys_all = [[None] * 4 for _ in range(2)]
                    filler += pv_norm_units(pp, pc, pexs, pexd, ys_all)
                    prevprev = (pp, pc, ys_all)
                filler += post_map.get(i, [])
                sc = scores(p, c, filler)
                prev = (p, c, sc[0], sc[1])
            # ----- tail: h-split the last item (3, 3) so the terminal
            # Act burst is halved, with the final out-projections as filler --
            fillerA = []
            qp, qc2, qys = prevprev
            fillerA += trans_copy_units(qp, qc2, qys)
            pp, pc, pexs, pexd = prev
            ys23 = [[None] * 4 for _ in range(2)]
            fillerA += pv_norm_units(pp, pc, pexs, pexd, ys23)
            fillerA.append(op_unit(7))
            scA = scores(3, 3, fillerA, hs=(0,))
            ys33 = [[None] * 4 for _ in range(2)]
            fillerB = trans_copy_units(pp, pc, ys23)
            fillerB += pv_norm_units(3, 3, scA[0], scA[1], ys33, hs=(0,))
            fillerB.append(op_unit(10))
            scB = scores(3, 3, fillerB, hs=(1,))
            outproj_tt(11)   # runs while the final exp burst drains
            for u in pv_norm_units(3, 3, scB[0], scB[1], ys33, hs=(1,)):
                u()
            tail_trans = trans_copy_units(3, 3, ys33)
            for qt_rel in range(4):
                tail_trans[qt_rel]()
                outproj_tt(12 + qt_rel)

    nc.compile()
    return nc


def _make_runner(nc):
    """Reusable 8-core SPMD runner (jit built once)."""
    import jax
    from jax.sharding import Mesh, PartitionSpec
    from jax.experimental.shard_map import shard_map
    from concourse import bass2jax
    import concourse.mybir as mybir

    bass2jax.install_neuronx_cc_hook()
    partition_name = (nc.partition_id_tensor.name
                      if nc.partition_id_tensor else None)
    in_names, out_names, out_avals, zero_outs = [], [], [], []
    for alloc in nc.m.functions[0].allocations:
        if not isinstance(alloc, mybir.MemoryLocationSet):
            continue
        name = alloc.memorylocations[0].name
        if alloc.kind == "ExternalInput":
            if name != partition_name:
                in_names.append(name)
        elif alloc.kind == "ExternalOutput":
            shape = tuple(alloc.tensor_shape)
            dtype = mybir.dt.np(alloc.dtype)
            out_names.append(name)
            out_avals.append(jax.core.ShapedArray(shape, dtype))
            zero_outs.append(np.zeros(shape, dtype))
    n_params = len(in_names)
    n_outs = len(out_avals)
    all_in = list(in_names) + list(out_names)
    if partition_name is not None:
        all_in.append(partition_name)

    def _body(*args):
        operands = list(args)
        if partition_name is not None:
            operands.append(bass2jax.partition_id_tensor())
        outs = bass2jax._bass_exec_p.bind(
            *operands,
            out_avals=tuple(out_avals),
            in_names=tuple(all_in),
            out_names=tuple(out_names),
            lowering_input_output_aliases=(),
            sim_require_finite=True,
            sim_require_nnan=True,
            nc=nc,
        )
        return tuple(outs)

    devices = jax.devices()[:N_CORES]
    mesh = Mesh(np.asarray(devices), ("core",))
    in_specs = (PartitionSpec("core"),) * (n_params + n_outs)
    out_specs = (PartitionSpec("core"),) * n_outs
    donate = tuple(range(n_params, n_params + n_outs))
    sharded = jax.jit(
        shard_map(_body, mesh=mesh, in_specs=in_specs, out_specs=out_specs,
                  check_rep=False),
        donate_argnums=donate, keep_unused=True)

    def run(in_maps):
        per_core = [[np.asarray(m[k]) for k in in_names] for m in in_maps]
        concat_in = [
            np.concatenate([per_core[c][i] for c in range(N_CORES)], axis=0)
            for i in range(n_params)]
        concat_zeros = [
            np.zeros((N_CORES * z.shape[0], *z.shape[1:]), z.dtype)
            for z in zero_outs]
        outs = sharded(*concat_in, *concat_zeros)
        jax.block_until_ready(outs)
        return [
            {name: np.asarray(outs[i]).reshape(N_CORES, *out_avals[i].shape)[c]
             for i, name in enumerate(out_names)}
            for c in range(N_CORES)]

    return run


def kernel(x, w_qkv, b_qkv, w_proj, b_proj):
    x = np.asarray(x, dtype=np.float32)
    w_qkv = np.asarray(w_qkv, dtype=np.float32)
    b_qkv = np.asarray(b_qkv, dtype=np.float32)
    w_proj = np.asarray(w_proj, dtype=np.float32)
    b_proj = np.asarray(b_proj, dtype=np.float32)

    w_q, w_k, w_v = w_qkv[0:1024], w_qkv[1024:2048], w_qkv[2048:3072]
    b_q, b_k, b_v = b_qkv[0:1024], b_qkv[1024:2048], b_qkv[2048:3072]
    has_qk_bias = bool(np.any(b_q) or np.any(b_k))

    key = ("runner", has_qk_bias)
    if key not in _RUNNER_CACHE:
        nc = _build(has_qk_bias)
        _RUNNER_CACHE[key] = _make_runner(nc)
    run = _RUNNER_CACHE[key]

    # causal block mask: tri2[k, j*128 + q] = 1.0 iff k <= q
    kk = np.arange(P)[:, None]
    qq = np.arange(P)[None, :]
    blk = (kk <= qq).astype(np.float16)
    tri2 = np.concatenate([blk, blk], axis=1)
    ident = np.eye(P, dtype=np.float16)

    in_maps = []
    for core in range(N_CORES):
        b, g = divmod(core, 2)
        xT_c = np.ascontiguousarray(x[b].T).astype(np.float16)
        if has_qk_bias:
            xT_c = np.concatenate([xT_c, np.ones((1, T), np.float16)], axis=0)
        wqk_c = np.empty((D_MODEL + (1 if has_qk_bias else 0), 1024),
                         np.float16)
        for p in range(PAIRS):
            hA = 8 * g + 2 * p
            hB = hA + 1
            cols = p * 256
            wqk_c[:D_MODEL, cols + 0:cols + 64] = w_q[hA * 64:(hA + 1) * 64].T
            wqk_c[:D_MODEL, cols + 64:cols + 128] = w_q[hB * 64:(hB + 1) * 64].T
            wqk_c[:D_MODEL, cols + 128:cols + 192] = w_k[hA * 64:(hA + 1) * 64].T
            wqk_c[:D_MODEL, cols + 192:cols + 256] = w_k[hB * 64:(hB + 1) * 64].T
            if has_qk_bias:
                wqk_c[D_MODEL, cols + 0:cols + 64] = b_q[hA * 64:(hA + 1) * 64]
                wqk_c[D_MODEL, cols + 64:cols + 128] = b_q[hB * 64:(hB + 1) * 64]
                wqk_c[D_MODEL, cols + 128:cols + 192] = b_k[hA * 64:(hA + 1) * 64]
                wqk_c[D_MODEL, cols + 192:cols + 256] = b_k[hB * 64:(hB + 1) * 64]
        wv_c = np.ascontiguousarray(
            w_v[8 * g * 64:(8 * g + 8) * 64].T).astype(np.float16)
        wp_c = np.ascontiguousarray(
            w_proj.T[g * 512:(g + 1) * 512, :]).astype(np.float16)
        in_maps.append({
            "xT": xT_c, "wqk": wqk_c, "wv": wv_c, "wp": wp_c,
            "tri2": tri2, "ident": ident,
        })

    results = run(in_maps)

    out = np.empty((B, T, D_MODEL), dtype=np.float32)
    for b in range(B):
        out[b] = (results[2 * b]["out"].astype(np.float32)
                  + results[2 * b + 1]["out"].astype(np.float32))

    # exact host-side bias folds (v-bias rides softmax row-sums == 1;
    # proj bias is additive)
    if np.any(b_v):
        out += (b_v @ w_proj.T)[None, None, :]
    if np.any(b_proj):
        out += b_proj[None, None, :]
    return out
